# revision 1
# baseline (speedup 1.0000x reference)
"""Low-rank (LoRA) linear for Trainium2, 8 NeuronCores.

Reference math:  out = x @ W^T + b + (ALPHA/R) * (x @ A^T) @ B^T
  x: (4, 2048, 4096) f32, W: (4096, 4096), b: (4096,), A: (16, 4096), B: (4096, 16)

Strategy:
  * Fold the adapter on the host: W_eff = W + SCALE * (B @ A).  The kernel is
    then a single dense GEMM  out = x @ W_eff^T + b.
  * Data-parallel over tokens: 8192 tokens -> 8 cores x 1024 tokens.
  * bf16 matmul (f32 PSUM accumulation).  Per core: M=1024, K=4096, N=4096
    -> 34.4 GFLOP, PE-bound at ~437 us (78.6 TF/s peak).
  * x^T kept SBUF-resident per core (8.4 MB bf16); W_eff^T streamed once in
    eight 4.2 MB column blocks, triple-buffered.
  * lhsT = x^T tile [128d, 128s] stationary; rhs = W_eff^T [128d, 512o]
    moving; 32 d-chunks accumulate into one PSUM bank; bias added on DVE
    during PSUM eviction.

All host-side prep (fold, transpose, bf16 cast, shard, gather) is numpy.
"""

import os

os.environ.setdefault("MYCRO_LOCAL_CACHE", "1")

import numpy as np
import ml_dtypes

R = 16
ALPHA = 32.0
SCALE = ALPHA / R

P = 128          # partitions
D = 4096         # d_in (contraction)
O = 4096         # d_out
S_FULL = 8192    # 4*2048 tokens
N_CORES = 8
S = S_FULL // N_CORES   # tokens per core
DO = D // P             # 32 contraction chunks
ST = S // P             # 8 token tiles per core
NB = 512                # output cols per matmul (one PSUM bank, f32)
OE = O // NB            # 8 output-column blocks

BF16 = ml_dtypes.bfloat16

_cache = {}


def _build_module():
    import concourse.mybir as mybir
    import concourse.tile as tile
    from concourse import bacc

    nc = bacc.Bacc(
        "TRN2", target_bir_lowering=False, debug=False, num_devices=N_CORES
    )
    xT = nc.dram_tensor("xT", (P, DO, S), mybir.dt.bfloat16, kind="ExternalInput").ap()
    wT = nc.dram_tensor(
        "wT", (OE, P, DO, NB), mybir.dt.bfloat16, kind="ExternalInput"
    ).ap()
    bb = nc.dram_tensor("bb", (P, O), mybir.dt.float32, kind="ExternalInput").ap()
    out = nc.dram_tensor("out", (S, O), mybir.dt.float32, kind="ExternalOutput").ap()

    with tile.TileContext(nc) as tc:
        with tc.tile_pool(name="xp", bufs=1) as xp, \
             tc.tile_pool(name="wp", bufs=3) as wp, \
             tc.tile_pool(name="bp", bufs=1) as bp, \
             tc.tile_pool(name="op", bufs=4) as op, \
             tc.tile_pool(name="pp", bufs=4, space="PSUM") as pp:

            x_sb = xp.tile([P, DO, S], mybir.dt.bfloat16)
            # split the x load so the first matmuls start after ~1 MB
            for st in range(ST):
                nc.sync.dma_start(
                    out=x_sb[:, :, st * P:(st + 1) * P],
                    in_=xT[:, :, st * P:(st + 1) * P],
                )
            b_sb = bp.tile([P, O], mybir.dt.float32)
            nc.sync.dma_start(out=b_sb[:], in_=bb[:])

            for oe in range(OE):
                w_sb = wp.tile([P, DO, NB], mybir.dt.bfloat16, tag="w")
                nc.sync.dma_start(out=w_sb[:], in_=wT[oe])
                for st in range(ST):
                    ps = pp.tile([P, NB], mybir.dt.float32, tag="ps")
                    for do in range(DO):
                        nc.tensor.matmul(
                            ps[:],
                            x_sb[:, do, st * P:(st + 1) * P],
                            w_sb[:, do, :],
                            start=(do == 0),
                            stop=(do == DO - 1),
                        )
                    o_sb = op.tile([P, NB], mybir.dt.float32, tag="o")
                    nc.vector.tensor_add(o_sb[:], ps[:], b_sb[:, oe * NB:(oe + 1) * NB])
                    nc.sync.dma_start(
                        out=out[st * P:(st + 1) * P, oe * NB:(oe + 1) * NB],
                        in_=o_sb[:],
                    )
    nc.compile()
    return nc


def _get_module():
    if "nc" not in _cache:
        _cache["nc"] = _build_module()
    return _cache["nc"]


def _prep_inputs(x, W, b, A, B):
    """Host-side: fold adapter, transpose to kernel layouts, cast, shard."""
    W_eff = W.astype(np.float32) + SCALE * (
        B.astype(np.float32) @ A.astype(np.float32)
    )
    # wT[oe, p, do, oo] = W_eff[oe*NB+oo, do*P+p]  (= W_eff^T in [K,N] tiles)
    wT = np.ascontiguousarray(
        W_eff.T.reshape(DO, P, OE, NB).transpose(2, 1, 0, 3)
    ).astype(BF16)
    bb = np.ascontiguousarray(
        np.broadcast_to(b.astype(np.float32), (P, O))
    )
    x2 = np.asarray(x, dtype=np.float32).reshape(S_FULL, D)
    in_maps = []
    for c in range(N_CORES):
        xc = x2[c * S:(c + 1) * S]                       # (S, D)
        # xT[p, do, s] = xc[s, do*P+p]
        xTc = np.ascontiguousarray(
            xc.reshape(S, DO, P).transpose(2, 1, 0)
        ).astype(BF16)
        in_maps.append({"xT": xTc, "wT": wT, "bb": bb})
    return in_maps


def run(x, W, b, A, B, trace=False, **spmd_kwargs):
    """Run the kernel; returns (full_output, BassKernelResults)."""
    from concourse import bass_utils

    nc = _get_module()
    in_maps = _prep_inputs(x, W, b, A, B)
    res = bass_utils.run_bass_kernel_spmd(
        nc, in_maps, core_ids=list(range(N_CORES)), trace=trace, **spmd_kwargs
    )
    outs = [res.results[c]["out"] for c in range(N_CORES)]
    full = np.concatenate(outs, axis=0).reshape(4, 2048, O)
    return full, res


def kernel(x, W, b, A, B):
    full, _ = run(x, W, b, A, B, trace=False)
    return full


# revision 5
# speedup vs baseline: 1.0173x; 1.0173x over previous
"""Low-rank (LoRA) linear for Trainium2, 8 NeuronCores.

Reference math:  out = x @ W^T + b + (ALPHA/R) * (x @ A^T) @ B^T
  x: (4, 2048, 4096) f32, W: (4096, 4096), b: (4096,), A: (16, 4096), B: (4096, 16)

Strategy:
  * Fold the adapter on the host: W_eff = W + SCALE * (B @ A).  The kernel is
    then a single dense GEMM  out = x @ W_eff^T + b.
  * Data-parallel over tokens: 8192 tokens -> 8 cores x 1024 tokens.
  * bf16 matmul (f32 PSUM accumulation).  Per core: M=1024, K=4096, N=4096
    -> 34.4 GFLOP, PE-bound at ~437 us (78.6 TF/s peak).
  * x^T kept SBUF-resident per core (8.4 MB bf16); W_eff^T streamed once in
    eight 4.2 MB column blocks, triple-buffered.
  * lhsT = x^T tile [128d, 128s] stationary; rhs = W_eff^T [128d, 512o]
    moving; 32 d-chunks accumulate into one PSUM bank; bias added on DVE
    during PSUM eviction.

All host-side prep (fold, transpose, bf16 cast, shard, gather) is numpy.
"""

import os

os.environ.setdefault("MYCRO_LOCAL_CACHE", "1")

import numpy as np
import ml_dtypes

R = 16
ALPHA = 32.0
SCALE = ALPHA / R

P = 128          # partitions
D = 4096         # d_in (contraction)
O = 4096         # d_out
S_FULL = 8192    # 4*2048 tokens
N_CORES = 8
S = S_FULL // N_CORES   # tokens per core
DO = D // P             # 32 contraction chunks
ST = S // P             # 8 token tiles per core
NB = 512                # output cols per matmul (one PSUM bank, f32)
OE = O // NB            # 8 output-column blocks

BF16 = ml_dtypes.bfloat16

_cache = {}


def _build_module():
    import concourse.mybir as mybir
    import concourse.tile as tile
    from concourse import bacc

    nc = bacc.Bacc(
        "TRN2", target_bir_lowering=False, debug=False, num_devices=N_CORES
    )
    xT = nc.dram_tensor("xT", (P, DO, S), mybir.dt.bfloat16, kind="ExternalInput").ap()
    wT = nc.dram_tensor(
        "wT", (OE, P, DO, NB), mybir.dt.bfloat16, kind="ExternalInput"
    ).ap()
    bb = nc.dram_tensor("bb", (P, O), mybir.dt.float32, kind="ExternalInput").ap()
    out = nc.dram_tensor("out", (S, O), mybir.dt.float32, kind="ExternalOutput").ap()

    DSUB = 8          # d-chunks per W sub-tile
    NSUB = DO // DSUB  # 4 sub-tiles per o-block

    with tile.TileContext(nc) as tc:
        with tc.tile_pool(name="xp", bufs=1) as xp, \
             tc.tile_pool(name="wp", bufs=3 * NSUB) as wp, \
             tc.tile_pool(name="bp", bufs=1) as bp, \
             tc.tile_pool(name="op", bufs=4) as op, \
             tc.tile_pool(name="pp", bufs=4, space="PSUM") as pp:

            # DMA_DIRECT2D occupies the issuing engine for the whole
            # transfer, so startup loads are spread over four engines and
            # split into per-chunk tiles (whole-tile dep granularity).
            engs = [nc.sync, nc.gpsimd, nc.scalar]

            def w_tiles(oe, issue):
                tiles = []
                for sub in range(NSUB):
                    t = wp.tile([P, DSUB, NB], mybir.dt.bfloat16, tag="w")
                    issue[sub % len(issue)].dma_start(
                        out=t[:], in_=wT[oe, :, sub * DSUB:(sub + 1) * DSUB, :]
                    )
                    tiles.append(t)
                return tiles

            # first o-block's weights: parallel across engines
            w_cur = w_tiles(0, engs)
            # x token-tiles: 8 chunks round-robin over the engines
            x_t = []
            for st in range(ST):
                t = xp.tile([P, DO, P], mybir.dt.bfloat16, tag=f"x{st}")
                engs[st % len(engs)].dma_start(out=t[:], in_=xT[:, :, st * P:(st + 1) * P])
                x_t.append(t)
            b_sb = bp.tile([P, O], mybir.dt.float32)
            nc.scalar.dma_start(out=b_sb[:], in_=bb[:])

            for oe in range(OE):
                w_nxt = w_tiles(oe + 1, [nc.sync, nc.gpsimd]) if oe + 1 < OE else None
                for st in range(ST):
                    ps = pp.tile([P, NB], mybir.dt.float32, tag="ps")
                    for do in range(DO):
                        nc.tensor.matmul(
                            ps[:],
                            x_t[st][:, do, :],
                            w_cur[do // DSUB][:, do % DSUB, :],
                            start=(do == 0),
                            stop=(do == DO - 1),
                        )
                    o_sb = op.tile([P, NB], mybir.dt.float32, tag="o")
                    nc.vector.tensor_add(o_sb[:], ps[:], b_sb[:, oe * NB:(oe + 1) * NB])
                    nc.scalar.dma_start(
                        out=out[st * P:(st + 1) * P, oe * NB:(oe + 1) * NB],
                        in_=o_sb[:],
                    )
                w_cur = w_nxt
    nc.compile()
    return nc


def _get_module():
    if "nc" not in _cache:
        _cache["nc"] = _build_module()
    return _cache["nc"]


def _prep_inputs(x, W, b, A, B):
    """Host-side: fold adapter, transpose to kernel layouts, cast, shard."""
    W_eff = W.astype(np.float32) + SCALE * (
        B.astype(np.float32) @ A.astype(np.float32)
    )
    # wT[oe, p, do, oo] = W_eff[oe*NB+oo, do*P+p]  (= W_eff^T in [K,N] tiles)
    wT = np.ascontiguousarray(
        W_eff.T.reshape(DO, P, OE, NB).transpose(2, 1, 0, 3)
    ).astype(BF16)
    bb = np.ascontiguousarray(
        np.broadcast_to(b.astype(np.float32), (P, O))
    )
    x2 = np.asarray(x, dtype=np.float32).reshape(S_FULL, D)
    in_maps = []
    for c in range(N_CORES):
        xc = x2[c * S:(c + 1) * S]                       # (S, D)
        # xT[p, do, s] = xc[s, do*P+p]
        xTc = np.ascontiguousarray(
            xc.reshape(S, DO, P).transpose(2, 1, 0)
        ).astype(BF16)
        in_maps.append({"xT": xTc, "wT": wT, "bb": bb})
    return in_maps


def run(x, W, b, A, B, trace=False, **spmd_kwargs):
    """Run the kernel; returns (full_output, BassKernelResults)."""
    from concourse import bass_utils

    nc = _get_module()
    in_maps = _prep_inputs(x, W, b, A, B)
    res = bass_utils.run_bass_kernel_spmd(
        nc, in_maps, core_ids=list(range(N_CORES)), trace=trace, **spmd_kwargs
    )
    outs = [res.results[c]["out"] for c in range(N_CORES)]
    full = np.concatenate(outs, axis=0).reshape(4, 2048, O)
    return full, res


def kernel(x, W, b, A, B):
    full, _ = run(x, W, b, A, B, trace=False)
    return full


# revision 8
# speedup vs baseline: 1.0599x; 1.0419x over previous
"""Low-rank (LoRA) linear for Trainium2, 8 NeuronCores.

Reference math:  out = x @ W^T + b + (ALPHA/R) * (x @ A^T) @ B^T
  x: (4, 2048, 4096) f32, W: (4096, 4096), b: (4096,), A: (16, 4096), B: (4096, 16)

Strategy:
  * Fold the adapter on the host: W_eff = W + SCALE * (B @ A).  The kernel is
    then a single dense GEMM  out = x @ W_eff^T + b.
  * Data-parallel over tokens: 8192 tokens -> 8 cores x 1024 tokens.
  * bf16 matmul (f32 PSUM accumulation).  Per core: M=1024, K=4096, N=4096
    -> 34.4 GFLOP, PE-bound at ~437 us (78.6 TF/s peak).
  * x^T kept SBUF-resident per core (8.4 MB bf16); W_eff^T streamed once in
    eight 4.2 MB column blocks, triple-buffered.
  * lhsT = x^T tile [128d, 128s] stationary; rhs = W_eff^T [128d, 512o]
    moving; 32 d-chunks accumulate into one PSUM bank; bias added on DVE
    during PSUM eviction.

All host-side prep (fold, transpose, bf16 cast, shard, gather) is numpy.
"""

import os

os.environ.setdefault("MYCRO_LOCAL_CACHE", "1")

import numpy as np
import ml_dtypes

R = 16
ALPHA = 32.0
SCALE = ALPHA / R

P = 128          # partitions
D = 4096         # d_in (contraction)
O = 4096         # d_out
S_FULL = 8192    # 4*2048 tokens
N_CORES = 8
S = S_FULL // N_CORES   # tokens per core
DO = D // P             # 32 contraction chunks
ST = S // P             # 8 token tiles per core
NB = 512                # output cols per matmul (one PSUM bank, f32)
OE = O // NB            # 8 output-column blocks

BF16 = ml_dtypes.bfloat16

_cache = {}


def _build_module():
    import concourse.mybir as mybir
    import concourse.tile as tile
    from concourse import bacc

    nc = bacc.Bacc(
        "TRN2", target_bir_lowering=False, debug=False, num_devices=N_CORES
    )
    xT = nc.dram_tensor(
        "xT", (ST, P, DO, P), mybir.dt.bfloat16, kind="ExternalInput"
    ).ap()
    wT = nc.dram_tensor(
        "wT", (OE, P, DO, NB), mybir.dt.bfloat16, kind="ExternalInput"
    ).ap()
    bb = nc.dram_tensor("bb", (P, O), mybir.dt.float32, kind="ExternalInput").ap()
    out = nc.dram_tensor("out", (S, O), mybir.dt.float32, kind="ExternalOutput").ap()

    DSUB = 8          # d-chunks per W sub-tile
    NSUB = DO // DSUB  # 4 sub-tiles per o-block

    with tile.TileContext(nc) as tc:
        with tc.tile_pool(name="xp", bufs=1) as xp, \
             tc.tile_pool(name="wp", bufs=3 * NSUB) as wp, \
             tc.tile_pool(name="bp", bufs=1) as bp, \
             tc.tile_pool(name="op", bufs=4) as op, \
             tc.tile_pool(name="pp", bufs=4, space="PSUM") as pp:

            # DMA_DIRECT2D occupies the issuing engine for the whole
            # transfer, so startup loads are spread over four engines and
            # split into per-chunk tiles (whole-tile dep granularity).
            engs = [nc.sync, nc.gpsimd, nc.scalar]

            def w_tiles(oe, issue):
                tiles = []
                for sub in range(NSUB):
                    t = wp.tile([P, DSUB, NB], mybir.dt.bfloat16, tag="w")
                    issue[sub % len(issue)].dma_start(
                        out=t[:], in_=wT[oe, :, sub * DSUB:(sub + 1) * DSUB, :]
                    )
                    tiles.append(t)
                return tiles

            # first o-block's weights: parallel across engines
            w_cur = w_tiles(0, engs)
            # x token-tiles: 8 chunks round-robin over the engines
            x_t = []
            for st in range(ST):
                t = xp.tile([P, DO, P], mybir.dt.bfloat16, tag=f"x{st}")
                engs[st % len(engs)].dma_start(out=t[:], in_=xT[st])
                x_t.append(t)
            b_sb = bp.tile([P, O], mybir.dt.float32)
            nc.scalar.dma_start(out=b_sb[:], in_=bb[:])

            for oe in range(OE):
                w_nxt = w_tiles(oe + 1, [nc.sync, nc.gpsimd]) if oe + 1 < OE else None
                for st in range(ST):
                    ps = pp.tile([P, NB], mybir.dt.float32, tag="ps")
                    for do in range(DO):
                        nc.tensor.matmul(
                            ps[:],
                            x_t[st][:, do, :],
                            w_cur[do // DSUB][:, do % DSUB, :],
                            start=(do == 0),
                            stop=(do == DO - 1),
                        )
                    o_sb = op.tile([P, NB], mybir.dt.float32, tag="o")
                    nc.vector.tensor_add(o_sb[:], ps[:], b_sb[:, oe * NB:(oe + 1) * NB])
                    nc.scalar.dma_start(
                        out=out[st * P:(st + 1) * P, oe * NB:(oe + 1) * NB],
                        in_=o_sb[:],
                    )
                w_cur = w_nxt
    nc.compile()
    return nc


def _get_module():
    if "nc" not in _cache:
        _cache["nc"] = _build_module()
    return _cache["nc"]


def _prep_inputs(x, W, b, A, B):
    """Host-side: fold adapter, transpose to kernel layouts, cast, shard."""
    W_eff = W.astype(np.float32) + SCALE * (
        B.astype(np.float32) @ A.astype(np.float32)
    )
    # wT[oe, p, do, oo] = W_eff[oe*NB+oo, do*P+p]  (= W_eff^T in [K,N] tiles)
    wT = np.ascontiguousarray(
        W_eff.T.reshape(DO, P, OE, NB).transpose(2, 1, 0, 3)
    ).astype(BF16)
    bb = np.ascontiguousarray(
        np.broadcast_to(b.astype(np.float32), (P, O))
    )
    x2 = np.asarray(x, dtype=np.float32).reshape(S_FULL, D)
    in_maps = []
    for c in range(N_CORES):
        xc = x2[c * S:(c + 1) * S]                       # (S, D)
        # xT[st, p, do, s'] = xc[st*P+s', do*P+p]  (contiguous per (st, p))
        xTc = np.ascontiguousarray(
            xc.reshape(ST, P, DO, P).transpose(0, 3, 2, 1)
        ).astype(BF16)
        in_maps.append({"xT": xTc, "wT": wT, "bb": bb})
    return in_maps


def run(x, W, b, A, B, trace=False, **spmd_kwargs):
    """Run the kernel; returns (full_output, BassKernelResults)."""
    from concourse import bass_utils

    nc = _get_module()
    in_maps = _prep_inputs(x, W, b, A, B)
    res = bass_utils.run_bass_kernel_spmd(
        nc, in_maps, core_ids=list(range(N_CORES)), trace=trace, **spmd_kwargs
    )
    outs = [res.results[c]["out"] for c in range(N_CORES)]
    full = np.concatenate(outs, axis=0).reshape(4, 2048, O)
    return full, res


def kernel(x, W, b, A, B):
    full, _ = run(x, W, b, A, B, trace=False)
    return full
